# revision 48
# baseline (speedup 1.0000x reference)
"""Trainium2 Bass kernel for single-token multi-head self-attention.

Problem (hardcoded):
  q: (1, 32, 512) f32, k/v: (8192, 32, 512) f32, 8 heads x 64 dim,
  scores = (q.k)/8, softcapped 10*tanh(.), softmax over klen, out = w.v.

Strategy: data-parallel over batch, 4 batches per core on 8 cores. The
problem is HBM-bandwidth bound. K/q are staged to device HBM as fp16 (half
the f32 traffic); V is staged as fp8 E3M4 (4 mantissa bits), each
(core, j) row of 2048 values pre-scaled so its absmax sits at 14 (inside
E3M4's [0.25, 15.5] normal range). Quantization noise ~1.3e-2 vs the 2e-2
budget. fp8 V feeds the PE matmul directly — no on-chip upcast — and the
row scale folds into the exp bias: e2 = exp(10*tanh(s) - ln(sf_j)), a
per-partition [P,1] bias (j is the partition axis).

Per core, K/V stream in j-chunks of J_FOLD*128 rows:
  - scores via DVE: k_t *= q_broadcast (fp16 2x mode), then a full fp16
    tree-halving add chain over d down to 1 (a segmented reduce only runs
    in 1x DVE mode and loses to the two extra small adds); ACT reads the
    strided column-0 scores directly
  - ACT: t = tanh(s/8); e1 = exp(10t) fp16 (softmax denominator),
    e2 = exp(10t + vln_j) fp16 (PV weights, undoes the V row scale)
  - P@V and sum(e1) accumulated on PE into fp32 PSUM across all chunks
    (lhsT = e2-slice (128,8) fp16, rhs = v-slice (128,512) fp8;
    ones column with e1 for the softmax denominator)
Notes from tried-and-rejected variants: GpSimd score offload regresses
(DVE+GP co-running contend on SBUF ports, inflating both ~40%); J_FOLD=4
regresses (longer serial per-block chains stall the pipeline more than
the amortized per-instruction fixed cost saves); int8 V with an on-chip
upcast regresses (any 1-byte operand demotes DVE to 1x mode and the cast
alone costs more than the DMA saving).

Epilogue ships the raw PV block (8, 4x512) and exp-sums (32,) to DRAM in
fp32; the tiny diagonal extraction out[b,h,:] = pv[h, b, h*64:] / s[b,h]
is done on the host (64 KB per core, negligible).
"""

import ml_dtypes
import numpy as np

import concourse.bass as bass
import concourse.bacc as bacc
import concourse.tile as tile
from concourse import mybir
from concourse.bass_utils import run_bass_kernel_spmd

N_CORES = 8
KLEN = 8192
BSZ = 32
D_MODEL = 512
N_HEAD = 8
D_HEAD = 64
B_PER_CORE = BSZ // N_CORES            # 4
BH = B_PER_CORE * N_HEAD               # 32
FREE = B_PER_CORE * D_MODEL            # 2048
P = 128                                # j rows per sub-chunk (partition dim)
J_FOLD = 2                             # sub-chunks folded per DMA/iteration
N_CHUNK = KLEN // P                    # 64
SCALE = 1.0 / D_HEAD**0.5              # 0.125
CLIP = 10.0
V_TARGET = 14.0                        # row absmax target after scaling

F8 = mybir.dt.float8e3
F16 = mybir.dt.float16
F32 = mybir.dt.float32
NP_F8 = ml_dtypes.float8_e3m4

_PROG_CACHE: dict = {}


def build_program(klen: int = KLEN):
    """Build the per-core Bass program (SPMD: same program, per-core data)."""
    rows = P * J_FOLD
    assert klen % rows == 0

    # Bacc (not plain Bass): its compile() pass splits multi-semaphore waits
    # into event-semaphore chains — TRN2 allows at most 1 wait per instruction.
    nc = bacc.Bacc()
    q_d = nc.dram_tensor("q", [1, FREE], F16, kind="ExternalInput")
    k_d = nc.dram_tensor("k", [klen, B_PER_CORE, D_MODEL], F16, kind="ExternalInput")
    # fp8e3 payload travels as uint8 (the jax/axon bridge rejects fp8 arrays);
    # the SBUF tile is bitcast back to float8e3 at the matmul.
    v_d = nc.dram_tensor(
        "v", [klen, B_PER_CORE, D_MODEL], mybir.dt.uint8, kind="ExternalInput"
    )
    vln_d = nc.dram_tensor("vln", [P, klen // P], F32, kind="ExternalInput")
    pv_d = nc.dram_tensor(
        "pv", [N_HEAD, B_PER_CORE * D_MODEL], F32, kind="ExternalOutput"
    )
    s_d = nc.dram_tensor("s", [BH, 1], F32, kind="ExternalOutput")

    with tile.TileContext(nc) as tc:
        with (
            tc.tile_pool(name="kv", bufs=6) as kv_pool,
            tc.tile_pool(name="small", bufs=3) as small_pool,
            tc.tile_pool(name="singles", bufs=1) as singles,
            tc.tile_pool(name="psum", bufs=1, space="PSUM") as psum_pool,
        ):
            # q replicated to all 128 partitions via broadcast DMA (SWDGE),
            # then fold-tiled on-chip (a stride-0 AP at the mul would demote
            # the DVE op to 1x mode)
            q_sb = singles.tile([P, J_FOLD, FREE], F16)
            q_ap = q_d[:]
            q_bcast = bass.AP(
                tensor=q_ap.tensor,
                offset=q_ap.offset,
                ap=[[0, P], list(q_ap.ap[-1])],
            )
            nc.gpsimd.dma_start(out=q_sb[:, 0, :], in_=q_bcast)
            for o in range(1, J_FOLD):
                nc.vector.tensor_copy(out=q_sb[:, o, :], in_=q_sb[:, 0, :])

            # -ln(v scale) per j, laid out [p, chunk] with j = chunk*128 + p
            vln_sb = singles.tile([P, klen // P], F32)
            nc.sync.dma_start(out=vln_sb[:], in_=vln_d[:])

            ones_sb = singles.tile([P, 1], F16)
            nc.vector.memset(ones_sb[:], 1.0)

            # persistent PSUM accumulators (a matmul's out must fit one PSUM
            # bank: <=512 fp32 per partition, so PV is 4 per-batch matmuls)
            pv_ps = [
                psum_pool.tile([N_HEAD, D_MODEL], F32, name=f"pv{b}")
                for b in range(B_PER_CORE)
            ]
            s_ps = psum_pool.tile([BH, 1], F32, name="s")

            kv_flat = k_d[:].rearrange("j b d -> j (b d)")
            vv_flat = v_d[:].rearrange("j b d -> j (b d)")

            # fold-2 blocks for the bulk; single-P blocks at the end so the
            # serial tail compute after the last DMA is as small as possible
            blocks = []
            j0 = 0
            while klen - j0 > 2 * P:
                blocks.append((j0, J_FOLD))
                j0 += J_FOLD * P
            while j0 < klen:
                blocks.append((j0, 1))
                j0 += P

            for bi, (j0, fold) in enumerate(blocks):
                k_t = kv_pool.tile([P, fold, FREE], F16, tag="k")
                v_t = kv_pool.tile([P, fold, FREE], mybir.dt.uint8, tag="v")
                k_src = kv_flat[j0 : j0 + fold * P].rearrange(
                    "(o p) f -> p o f", p=P
                )
                v_src = vv_flat[j0 : j0 + fold * P].rearrange(
                    "(o p) f -> p o f", p=P
                )
                # K on the SP HWDGE ring, V on the ACT HWDGE ring — the two
                # physical rings run concurrently, hiding per-DMA ramp
                nc.sync.dma_start(out=k_t[:], in_=k_src)
                nc.scalar.dma_start(out=v_t[:], in_=v_src)

                # scores: k_t *= q (in place, fp16 2x mode), then fp16
                # tree-halving to 4 and a final fp32 segmented reduce.
                # (Tree-to-1 with strided ACT reads was tried: slightly
                # slower than this shape.)
                nc.vector.tensor_mul(
                    out=k_t[:], in0=k_t[:], in1=q_sb[:, 0:fold, :]
                )
                kd = k_t[:].rearrange("p o (g d) -> p o g d", d=D_HEAD)
                for w in (32, 16, 8, 4):
                    nc.vector.tensor_add(
                        out=kd[:, :, :, 0:w], in0=kd[:, :, :, 0:w],
                        in1=kd[:, :, :, w : 2 * w],
                    )
                sc = small_pool.tile([P, fold, BH], F32, tag="sc")
                nc.vector.reduce_sum(
                    out=sc[:], in_=kd[:, :, :, 0:4], axis=mybir.AxisListType.X
                )

                # softcap + exp on ACT. e1 = exp(10 t) for the denominator,
                # e2 = exp(10 t + vln_j) for PV (undoes the V row scale;
                # vln <= -0.8 so e2 <= exp(9.2) fits fp16).
                nc.scalar.activation(
                    out=sc[:], in_=sc[:],
                    func=mybir.ActivationFunctionType.Tanh, scale=SCALE,
                )
                e1 = small_pool.tile([P, fold, BH], F16, tag="e1")
                nc.scalar.activation(
                    out=e1[:], in_=sc[:],
                    func=mybir.ActivationFunctionType.Exp, scale=CLIP,
                )
                e2 = small_pool.tile([P, fold, BH], F16, tag="e2")
                for o in range(fold):
                    nc.scalar.activation(
                        out=e2[:, o, :],
                        in_=sc[:, o, :],
                        func=mybir.ActivationFunctionType.Exp, scale=CLIP,
                        bias=vln_sb[:, j0 // P + o : j0 // P + o + 1],
                    )

                start = bi == 0
                stop = bi == len(blocks) - 1
                for o in range(fold):
                    for b in range(B_PER_CORE):
                        nc.tensor.matmul(
                            pv_ps[b][:],
                            lhsT=e2[:, o, b * N_HEAD : (b + 1) * N_HEAD],
                            rhs=v_t[:, o, b * D_MODEL : (b + 1) * D_MODEL].bitcast(F8),
                            start=start and o == 0,
                            stop=stop and o == fold - 1,
                        )
                    nc.tensor.matmul(
                        s_ps[:],
                        lhsT=e1[:, o, :],
                        rhs=ones_sb[:],
                        start=start and o == 0,
                        stop=stop and o == fold - 1,
                    )

            # epilogue: PSUM -> SBUF -> DRAM (fp32). The tiny s chain goes
            # first and out on the ACT ring so its DMA fixed latency overlaps
            # the pv DMA on the SP ring; pv copies split over ACT+DVE.
            s_sb = singles.tile([BH, 1], F32)
            nc.vector.tensor_copy(out=s_sb[:], in_=s_ps[:])
            nc.scalar.dma_start(out=s_d[:], in_=s_sb[:])
            pv_sb = singles.tile([N_HEAD, B_PER_CORE * D_MODEL], F32)
            for b in range(B_PER_CORE):
                out_slice = pv_sb[:, b * D_MODEL : (b + 1) * D_MODEL]
                if b % 2 == 0:
                    nc.scalar.copy(out=out_slice, in_=pv_ps[b][:])
                else:
                    nc.vector.tensor_copy(out=out_slice, in_=pv_ps[b][:])
            nc.sync.dma_start(out=pv_d[:], in_=pv_sb[:])
    nc.finalize()
    return nc


def shard_inputs(q: np.ndarray, k: np.ndarray, v: np.ndarray):
    """Split full inputs into per-core input maps (fp16 K/q, fp8e3 V)."""
    q16 = np.asarray(q, dtype=np.float16)
    k16 = np.asarray(k, dtype=np.float16)
    v32 = np.asarray(v, dtype=np.float32)
    in_maps = []
    for i in range(N_CORES):
        b0 = i * B_PER_CORE
        vc = np.ascontiguousarray(v32[:, b0 : b0 + B_PER_CORE, :]).reshape(
            KLEN, FREE
        )
        vmax = np.maximum(np.abs(vc).max(axis=1), 1e-20)
        sf = V_TARGET / vmax
        v8 = np.clip(vc * sf[:, None], -15.0, 15.0).astype(NP_F8).view(np.uint8)
        # vln[p, c] pairs with j = c*128 + p
        vln = (-np.log(sf)).astype(np.float32).reshape(N_CHUNK, P).T
        in_maps.append(
            {
                "q": np.ascontiguousarray(
                    q16[0, b0 : b0 + B_PER_CORE, :]
                ).reshape(1, FREE),
                "k": np.ascontiguousarray(k16[:, b0 : b0 + B_PER_CORE, :]),
                "v": v8.reshape(KLEN, B_PER_CORE, D_MODEL),
                "vln": np.ascontiguousarray(vln),
            }
        )
    return in_maps


def combine_outputs(results) -> np.ndarray:
    """Per-core (pv, s) -> full (1, 32, 512): diagonal extract + normalize."""
    outs = []
    hh = np.arange(N_HEAD)
    for i in range(N_CORES):
        pv = np.asarray(results[i]["pv"], dtype=np.float32)
        s = np.asarray(results[i]["s"], dtype=np.float32).reshape(
            B_PER_CORE, N_HEAD
        )
        pv4 = pv.reshape(N_HEAD, B_PER_CORE, N_HEAD, D_HEAD)
        diag = pv4[hh, :, hh, :]          # (n_head, b, d_head), row h = head h
        o = diag.transpose(1, 0, 2)       # (b, h, d)
        o = o / s[:, :, None]
        outs.append(o.reshape(B_PER_CORE, D_MODEL))
    return np.concatenate(outs, axis=0)[None, :, :].astype(np.float32)


def kernel(q, k, v):
    q = np.asarray(q, dtype=np.float32)
    k = np.asarray(k, dtype=np.float32)
    v = np.asarray(v, dtype=np.float32)
    assert q.shape == (1, BSZ, D_MODEL) and k.shape == (KLEN, BSZ, D_MODEL)

    if "prog" not in _PROG_CACHE:
        _PROG_CACHE["prog"] = build_program(KLEN)
    nc = _PROG_CACHE["prog"]

    in_maps = shard_inputs(q, k, v)
    res = run_bass_kernel_spmd(nc, in_maps, list(range(N_CORES))).results
    return combine_outputs(res)


if __name__ == "__main__":
    rng = np.random.default_rng(0)
    q = rng.standard_normal((1, BSZ, D_MODEL), dtype=np.float32)
    k = rng.standard_normal((KLEN, BSZ, D_MODEL), dtype=np.float32)
    v = rng.standard_normal((KLEN, BSZ, D_MODEL), dtype=np.float32)
    out = kernel(q, k, v)
    print(out.shape, out.dtype)


# revision 49
# speedup vs baseline: 1.1511x; 1.1511x over previous
"""Trainium2 Bass kernel for single-token multi-head self-attention.

Problem (hardcoded):
  q: (1, 32, 512) f32, k/v: (8192, 32, 512) f32, 8 heads x 64 dim,
  scores = (q.k)/8, softcapped 10*tanh(.), softmax over klen, out = w.v.

Strategy: data-parallel over batch, 4 batches per core on 8 cores. The
problem is HBM-bandwidth bound. K/q are staged to device HBM as fp16 (half
the f32 traffic); V is staged as fp8 E3M4 (4 mantissa bits), each
(core, j) row of 2048 values pre-scaled so its absmax sits at 14 (inside
E3M4's [0.25, 15.5] normal range). Quantization noise ~1.3e-2 vs the 2e-2
budget. fp8 V feeds the PE matmul directly — no on-chip upcast — and the
row scale folds into the exp bias: e2 = exp(10*tanh(s) - ln(sf_j)), a
per-partition [P,1] bias (j is the partition axis).

Per core, K/V stream in j-chunks of J_FOLD*128 rows:
  - scores via DVE: k_t *= q_broadcast (fp16 2x mode), then a full fp16
    tree-halving add chain over d down to 1 (a segmented reduce only runs
    in 1x DVE mode and loses to the two extra small adds); ACT reads the
    strided column-0 scores directly
  - ACT: t = tanh(s/8); e1 = exp(10t) fp16 (softmax denominator),
    e2 = exp(10t + vln_j) fp16 (PV weights, undoes the V row scale)
  - P@V and sum(e1) accumulated on PE into fp32 PSUM across all chunks
    (lhsT = e2-slice (128,8) fp16, rhs = v-slice (128,512) fp8;
    ones column with e1 for the softmax denominator)
Notes from tried-and-rejected variants: GpSimd score offload regresses
(DVE+GP co-running contend on SBUF ports, inflating both ~40%); J_FOLD=4
regresses (longer serial per-block chains stall the pipeline more than
the amortized per-instruction fixed cost saves); int8 V with an on-chip
upcast regresses (any 1-byte operand demotes DVE to 1x mode and the cast
alone costs more than the DMA saving).

Epilogue ships the raw PV block (8, 4x512) and exp-sums (32,) to DRAM in
fp32; the tiny diagonal extraction out[b,h,:] = pv[h, b, h*64:] / s[b,h]
is done on the host (64 KB per core, negligible).
"""

import ml_dtypes
import numpy as np

import concourse.bass as bass
import concourse.bacc as bacc
import concourse.tile as tile
from concourse import mybir
from concourse.bass_utils import run_bass_kernel_spmd

N_CORES = 8
KLEN = 8192
BSZ = 32
D_MODEL = 512
N_HEAD = 8
D_HEAD = 64
B_PER_CORE = BSZ // N_CORES            # 4
BH = B_PER_CORE * N_HEAD               # 32
FREE = B_PER_CORE * D_MODEL            # 2048
P = 128                                # j rows per sub-chunk (partition dim)
J_FOLD = 2                             # sub-chunks folded per DMA/iteration
N_CHUNK = KLEN // P                    # 64
SCALE = 1.0 / D_HEAD**0.5              # 0.125
CLIP = 10.0
V_TARGET = 14.0                        # row absmax target after scaling

F8 = mybir.dt.float8e3
F16 = mybir.dt.float16
F32 = mybir.dt.float32
NP_F8 = ml_dtypes.float8_e3m4

_PROG_CACHE: dict = {}


def build_program(klen: int = KLEN):
    """Build the per-core Bass program (SPMD: same program, per-core data)."""
    rows = P * J_FOLD
    assert klen % rows == 0

    # Bacc (not plain Bass): its compile() pass splits multi-semaphore waits
    # into event-semaphore chains — TRN2 allows at most 1 wait per instruction.
    nc = bacc.Bacc()
    q_d = nc.dram_tensor("q", [1, FREE], F16, kind="ExternalInput")
    k_d = nc.dram_tensor("k", [klen, B_PER_CORE, D_MODEL], F16, kind="ExternalInput")
    # fp8e3 payload travels as uint8 (the jax/axon bridge rejects fp8 arrays);
    # the SBUF tile is bitcast back to float8e3 at the matmul.
    v_d = nc.dram_tensor(
        "v", [klen, B_PER_CORE, D_MODEL], mybir.dt.uint8, kind="ExternalInput"
    )
    vln_d = nc.dram_tensor("vln", [P, klen // P], F32, kind="ExternalInput")
    pv_d = nc.dram_tensor(
        "pv", [N_HEAD, B_PER_CORE * D_MODEL], F32, kind="ExternalOutput"
    )
    s_d = nc.dram_tensor("s", [BH, 1], F32, kind="ExternalOutput")

    with tile.TileContext(nc) as tc:
        with (
            tc.tile_pool(name="kv", bufs=6) as kv_pool,
            tc.tile_pool(name="small", bufs=3) as small_pool,
            tc.tile_pool(name="singles", bufs=1) as singles,
            tc.tile_pool(name="psum", bufs=1, space="PSUM") as psum_pool,
        ):
            # q replicated to all 128 partitions via broadcast DMA (SWDGE),
            # then fold-tiled on-chip (a stride-0 AP at the mul would demote
            # the DVE op to 1x mode)
            q_sb = singles.tile([P, J_FOLD, FREE], F16)
            q_ap = q_d[:]
            q_bcast = bass.AP(
                tensor=q_ap.tensor,
                offset=q_ap.offset,
                ap=[[0, P], list(q_ap.ap[-1])],
            )
            nc.gpsimd.dma_start(out=q_sb[:, 0, :], in_=q_bcast)
            for o in range(1, J_FOLD):
                nc.vector.tensor_copy(out=q_sb[:, o, :], in_=q_sb[:, 0, :])

            # -ln(v scale) per j, laid out [p, chunk] with j = chunk*128 + p
            vln_sb = singles.tile([P, klen // P], F32)
            nc.sync.dma_start(out=vln_sb[:], in_=vln_d[:])

            ones_sb = singles.tile([P, 1], F16)
            nc.vector.memset(ones_sb[:], 1.0)

            # persistent PSUM accumulators (a matmul's out must fit one PSUM
            # bank: <=512 fp32 per partition, so PV is 4 per-batch matmuls)
            pv_ps = [
                psum_pool.tile([N_HEAD, D_MODEL], F32, name=f"pv{b}")
                for b in range(B_PER_CORE)
            ]
            s_ps = psum_pool.tile([BH, 1], F32, name="s")

            kv_flat = k_d[:].rearrange("j b d -> j (b d)")
            vv_flat = v_d[:].rearrange("j b d -> j (b d)")

            # fold-2 blocks for the bulk; single-P blocks at the end so the
            # serial tail compute after the last DMA is as small as possible
            blocks = []
            j0 = 0
            while klen - j0 > 2 * P:
                blocks.append((j0, J_FOLD))
                j0 += J_FOLD * P
            while j0 < klen:
                blocks.append((j0, 1))
                j0 += P

            for bi, (j0, fold) in enumerate(blocks):
                k_t = kv_pool.tile([P, fold, FREE], F16, tag="k")
                v_t = kv_pool.tile([P, fold, FREE], mybir.dt.uint8, tag="v")
                k_src = kv_flat[j0 : j0 + fold * P].rearrange(
                    "(o p) f -> p o f", p=P
                )
                v_src = vv_flat[j0 : j0 + fold * P].rearrange(
                    "(o p) f -> p o f", p=P
                )
                # spread the load across three DMA queues: K alternates
                # between the SP HWDGE ring and the gpsimd SWDGE ring, V on
                # the ACT HWDGE ring — each queue then carries ~17 MB, which
                # helps when HBM is contended (queues have DRAM-channel
                # affinity) and is neutral when DVE-bound
                k_eng = nc.sync if bi % 2 == 0 else nc.gpsimd
                k_eng.dma_start(out=k_t[:], in_=k_src)
                nc.scalar.dma_start(out=v_t[:], in_=v_src)

                # scores: k_t *= q (in place, fp16 2x mode), then fp16
                # tree-halving to 4 and a final fp32 segmented reduce.
                # (Tree-to-1 with strided ACT reads was tried: slightly
                # slower than this shape.)
                nc.vector.tensor_mul(
                    out=k_t[:], in0=k_t[:], in1=q_sb[:, 0:fold, :]
                )
                kd = k_t[:].rearrange("p o (g d) -> p o g d", d=D_HEAD)
                for w in (32, 16, 8, 4):
                    nc.vector.tensor_add(
                        out=kd[:, :, :, 0:w], in0=kd[:, :, :, 0:w],
                        in1=kd[:, :, :, w : 2 * w],
                    )
                sc = small_pool.tile([P, fold, BH], F32, tag="sc")
                nc.vector.reduce_sum(
                    out=sc[:], in_=kd[:, :, :, 0:4], axis=mybir.AxisListType.X
                )

                # softcap + exp on ACT. e1 = exp(10 t) for the denominator,
                # e2 = exp(10 t + vln_j) for PV (undoes the V row scale;
                # vln <= -0.8 so e2 <= exp(9.2) fits fp16).
                nc.scalar.activation(
                    out=sc[:], in_=sc[:],
                    func=mybir.ActivationFunctionType.Tanh, scale=SCALE,
                )
                e1 = small_pool.tile([P, fold, BH], F16, tag="e1")
                nc.scalar.activation(
                    out=e1[:], in_=sc[:],
                    func=mybir.ActivationFunctionType.Exp, scale=CLIP,
                )
                e2 = small_pool.tile([P, fold, BH], F16, tag="e2")
                for o in range(fold):
                    nc.scalar.activation(
                        out=e2[:, o, :],
                        in_=sc[:, o, :],
                        func=mybir.ActivationFunctionType.Exp, scale=CLIP,
                        bias=vln_sb[:, j0 // P + o : j0 // P + o + 1],
                    )

                start = bi == 0
                stop = bi == len(blocks) - 1
                for o in range(fold):
                    for b in range(B_PER_CORE):
                        nc.tensor.matmul(
                            pv_ps[b][:],
                            lhsT=e2[:, o, b * N_HEAD : (b + 1) * N_HEAD],
                            rhs=v_t[:, o, b * D_MODEL : (b + 1) * D_MODEL].bitcast(F8),
                            start=start and o == 0,
                            stop=stop and o == fold - 1,
                        )
                    nc.tensor.matmul(
                        s_ps[:],
                        lhsT=e1[:, o, :],
                        rhs=ones_sb[:],
                        start=start and o == 0,
                        stop=stop and o == fold - 1,
                    )

            # epilogue: PSUM -> SBUF -> DRAM (fp32). The tiny s chain goes
            # first and out on the ACT ring so its DMA fixed latency overlaps
            # the pv DMA on the SP ring; pv copies split over ACT+DVE.
            s_sb = singles.tile([BH, 1], F32)
            nc.vector.tensor_copy(out=s_sb[:], in_=s_ps[:])
            nc.scalar.dma_start(out=s_d[:], in_=s_sb[:])
            pv_sb = singles.tile([N_HEAD, B_PER_CORE * D_MODEL], F32)
            for b in range(B_PER_CORE):
                out_slice = pv_sb[:, b * D_MODEL : (b + 1) * D_MODEL]
                if b % 2 == 0:
                    nc.scalar.copy(out=out_slice, in_=pv_ps[b][:])
                else:
                    nc.vector.tensor_copy(out=out_slice, in_=pv_ps[b][:])
            nc.sync.dma_start(out=pv_d[:], in_=pv_sb[:])
    nc.finalize()
    return nc


def shard_inputs(q: np.ndarray, k: np.ndarray, v: np.ndarray):
    """Split full inputs into per-core input maps (fp16 K/q, fp8e3 V)."""
    q16 = np.asarray(q, dtype=np.float16)
    k16 = np.asarray(k, dtype=np.float16)
    v32 = np.asarray(v, dtype=np.float32)
    in_maps = []
    for i in range(N_CORES):
        b0 = i * B_PER_CORE
        vc = np.ascontiguousarray(v32[:, b0 : b0 + B_PER_CORE, :]).reshape(
            KLEN, FREE
        )
        vmax = np.maximum(np.abs(vc).max(axis=1), 1e-20)
        sf = V_TARGET / vmax
        v8 = np.clip(vc * sf[:, None], -15.0, 15.0).astype(NP_F8).view(np.uint8)
        # vln[p, c] pairs with j = c*128 + p
        vln = (-np.log(sf)).astype(np.float32).reshape(N_CHUNK, P).T
        in_maps.append(
            {
                "q": np.ascontiguousarray(
                    q16[0, b0 : b0 + B_PER_CORE, :]
                ).reshape(1, FREE),
                "k": np.ascontiguousarray(k16[:, b0 : b0 + B_PER_CORE, :]),
                "v": v8.reshape(KLEN, B_PER_CORE, D_MODEL),
                "vln": np.ascontiguousarray(vln),
            }
        )
    return in_maps


def combine_outputs(results) -> np.ndarray:
    """Per-core (pv, s) -> full (1, 32, 512): diagonal extract + normalize."""
    outs = []
    hh = np.arange(N_HEAD)
    for i in range(N_CORES):
        pv = np.asarray(results[i]["pv"], dtype=np.float32)
        s = np.asarray(results[i]["s"], dtype=np.float32).reshape(
            B_PER_CORE, N_HEAD
        )
        pv4 = pv.reshape(N_HEAD, B_PER_CORE, N_HEAD, D_HEAD)
        diag = pv4[hh, :, hh, :]          # (n_head, b, d_head), row h = head h
        o = diag.transpose(1, 0, 2)       # (b, h, d)
        o = o / s[:, :, None]
        outs.append(o.reshape(B_PER_CORE, D_MODEL))
    return np.concatenate(outs, axis=0)[None, :, :].astype(np.float32)


def kernel(q, k, v):
    q = np.asarray(q, dtype=np.float32)
    k = np.asarray(k, dtype=np.float32)
    v = np.asarray(v, dtype=np.float32)
    assert q.shape == (1, BSZ, D_MODEL) and k.shape == (KLEN, BSZ, D_MODEL)

    if "prog" not in _PROG_CACHE:
        _PROG_CACHE["prog"] = build_program(KLEN)
    nc = _PROG_CACHE["prog"]

    in_maps = shard_inputs(q, k, v)
    res = run_bass_kernel_spmd(nc, in_maps, list(range(N_CORES))).results
    return combine_outputs(res)


if __name__ == "__main__":
    rng = np.random.default_rng(0)
    q = rng.standard_normal((1, BSZ, D_MODEL), dtype=np.float32)
    k = rng.standard_normal((KLEN, BSZ, D_MODEL), dtype=np.float32)
    v = rng.standard_normal((KLEN, BSZ, D_MODEL), dtype=np.float32)
    out = kernel(q, k, v)
    print(out.shape, out.dtype)
